# revision 24
# baseline (speedup 1.0000x reference)
"""Trainium2 Bass kernel for nn_Decoder_48859547959519.

Autoregressive LSTM decoder: 512 sequential steps, batch 8, hidden 256,
feedback y_t = fc(h_{t+1}) -> x_{t+1}.

Strategy: data parallel (1 batch element per NeuronCore, 8 cores) +
**parallel-in-time fixed-point iteration** instead of a serial 512-step
loop.

  * Algebraic fusion: x_{t+1} = W_fc h_{t+1} + b_fc  =>  for t >= 1
        gates_t = (W_ih W_fc + W_hh) h_t + (W_ih b_fc + b) = W_eff h_t + b_eff
    so each trajectory position needs one 256->1024 matvec + LSTM cell.
    Step 0 (x_0 = 0) is peeled on the host.
  * The whole trajectory H = [h_1 .. h_512] is iterated as a fixed point:
        gates^k  = W_eff H^{k-1}(shifted) + b     (16 batched N=511 matmuls)
        i,f,o,g  = sigmoid/tanh(gates^k)          (8 big ACT ops, per-chunk
                                                   per-partition bias = free)
        c^k      = exact scan: c_t = f_t*c_{t-1} + i_t*tanh(g_t)
                                                  (DVE tensor_tensor_scan!)
        H^k      = o^k * tanh(c^k)
    Given the gates, the c-recurrence is solved EXACTLY within a sweep by
    the hardware prefix-scan; only the h-feedback lags one sweep.  The
    step map is strongly contractive (err ~0.65x/sweep for pure Jacobi,
    far faster with the exact c-scan): measured convergence to the bf16
    noise floor (~2e-3) in 4 sweeps; NSWEEP=6 leaves margin.  Positions
    t <= k are exact after k sweeps regardless.
  * Every op is a big batched op (N=511..1022) so fixed instruction
    overheads amortize; there is no per-timestep serial chain at all.
"""

import numpy as np

SEQ_LEN = 512
IN_DIM = 23
HID = 256
FEAT = 128
BATCH = 8
NCHUNK = 8  # 4*HID / 128
# chunk order: [g0 g1 i0 i1 f0 f1 o0 o1]
# (PyTorch gate-row order in W_eff is i:0 f:256 g:512 o:768)
CHUNK_ROWS = [512, 640, 0, 128, 256, 384, 768, 896]
# bank emission order within a sweep: g0, i0, i1, f0, g1, f1, o0, o1
MM_ORDER = [0, 2, 3, 4, 1, 5, 6, 7]
NSWEEP = 2

_CACHE = {}


def _sigmoid(x):
    return 1.0 / (1.0 + np.exp(-x))


def _host_prep(feature, W_ih, W_hh, b_ih, b_hh, W_fc, b_fc, W_hfc, b_hfc):
    """Fuse the feedback path, peel step 0, pack device tensors."""
    f32 = np.float32
    W_ih = np.asarray(W_ih, f32)
    W_hh = np.asarray(W_hh, f32)
    W_fc = np.asarray(W_fc, f32)
    b = np.asarray(b_ih, f32) + np.asarray(b_hh, f32)

    W_eff = (W_ih @ W_fc + W_hh).astype(f32)          # [1024, 256]
    b_eff = (W_ih @ np.asarray(b_fc, f32) + b).astype(f32)  # [1024]

    # step 0 on host (x_0 = 0): h0 from feature, c0 = 0
    feats = np.asarray(feature, f32)                  # [B, FEAT]
    h0 = feats @ np.asarray(W_hfc, f32).T + np.asarray(b_hfc, f32)  # [B, HID]
    g0 = h0 @ W_hh.T + b                              # [B, 1024]
    i_g, f_g, g_g, o_g = np.split(g0, 4, axis=1)
    c1 = _sigmoid(i_g) * np.tanh(g_g)                 # [B, HID]
    h1 = _sigmoid(o_g) * np.tanh(c1)                  # [B, HID]

    # pack weight tiles bank-major in matmul issue order (g0,i0,i1,f0,
    # g1,f1,o0,o1), both k-tiles of a bank adjacent, so the upload can be
    # split into two contiguous DMAs that deliver the first banks early:
    # wt[p, q*256 + k*128 + j] = W_eff[row(MM_ORDER[q])+j, k*128+p]
    wt = np.empty((128, 2048), np.float32)
    for q, m in enumerate(MM_ORDER):
        for k in range(2):
            blk = W_eff[CHUNK_ROWS[m]:CHUNK_ROWS[m] + 128,
                        k * 128:(k + 1) * 128]        # [j, p]
            wt[:, q * 256 + k * 128:q * 256 + (k + 1) * 128] = blk.T
    # per-chunk bias as [128, 8] per-partition vectors (ACT bias operand)
    bias_sb = np.stack([b_eff[r:r + 128] for r in CHUNK_ROWS], 1)  # [128, 8]

    # fc weights for the output stage: wfc[p, k*23+d] = W_fc[d, k*128+p]
    wfc = np.empty((128, 2 * IN_DIM), np.float32)
    for k in range(2):
        wfc[:, k * IN_DIM:(k + 1) * IN_DIM] = W_fc[:, k * 128:(k + 1) * 128].T
    bfc = np.asarray(b_fc, f32).reshape(IN_DIM, 1)

    import ml_dtypes
    bf16 = ml_dtypes.bfloat16
    T = SEQ_LEN
    per_core = []
    for bb in range(BATCH):
        # Initial guess = device sweep 1 computed in closed form on the host:
        # H^0 is zero except position 0 (= h1), so sweep-1 gates are
        # W_eff h1 + b at position 1 and plain b elsewhere -- one matvec
        # plus a scalar recurrence.  (Equivalent to one device sweep, in
        # fp32; the device then runs NSWEEP real sweeps on top.)
        H0 = np.zeros((128, 2 * T), np.float32)
        H0[:, 0] = h1[bb, 0:128]
        H0[:, T] = h1[bb, 128:256]
        c1p = np.stack([c1[bb, 0:128], c1[bb, 128:256]], 1)  # [128, 2]
        g1v = W_eff @ h1[bb] + b_eff                  # gates at position 1
        gbv = b_eff                                   # gates at positions >= 2
        ii, ff, gg, oo = (slice(0, 256), slice(256, 512),
                          slice(512, 768), slice(768, 1024))
        u1 = _sigmoid(g1v[ii]) * np.tanh(g1v[gg])
        ub = _sigmoid(gbv[ii]) * np.tanh(gbv[gg])
        f1 = _sigmoid(g1v[ff])
        fb = _sigmoid(gbv[ff])
        o1 = _sigmoid(g1v[oo])
        ob = _sigmoid(gbv[oo])
        cj = np.concatenate([c1p[:, 0], c1p[:, 1]])   # c at position 0
        Hf = np.zeros((256, T), np.float32)
        Hf[:, 0] = h1[bb]
        for t in range(1, T):
            cj = (f1 if t == 1 else fb) * cj + (u1 if t == 1 else ub)
            Hf[:, t] = (o1 if t == 1 else ob) * np.tanh(cj)
        H0[:, 0:T] = Hf[0:128]
        H0[:, T:2 * T] = Hf[128:256]
        per_core.append({
            "wt": wt.astype(bf16),
            "bias": bias_sb.astype(f32),
            "boT": np.concatenate([b_eff[768:896], b_eff[896:1024]]
                                  ).reshape(1, 256).astype(bf16),
            "biT": np.concatenate([b_eff[0:128], b_eff[128:256]]
                                  ).reshape(1, 256).astype(bf16),
            "wfc": wfc.astype(bf16),
            "H0": H0.astype(bf16),
            "c1a": np.ascontiguousarray(c1p[:, 0:1]),
            "c1b": np.ascontiguousarray(c1p[:, 1:2]),
        })
    return per_core


def build_program(T=SEQ_LEN, nsweep=NSWEEP):
    """Emit the Bass/Tile program (fully static, no hardware loop)."""
    import concourse.bacc as bacc
    import concourse.mybir as mybir
    import concourse.tile as tile

    f32 = mybir.dt.float32
    bf16 = mybir.dt.bfloat16
    SIG = mybir.ActivationFunctionType.Sigmoid
    TANH = mybir.ActivationFunctionType.Tanh
    IDT = mybir.ActivationFunctionType.Identity
    ALU = mybir.AluOpType

    N = T - 1  # positions computed per sweep (pos 1..T-1); pos 0 fixed
    nc = bacc.Bacc("TRN2", target_bir_lowering=False, debug=False)

    # DRAM I/O
    wt_d = nc.dram_tensor("wt", [128, 2048], bf16, kind="ExternalInput")
    bias_d = nc.dram_tensor("bias", [128, 8], f32, kind="ExternalInput")
    boT_d = nc.dram_tensor("boT", [1, 256], bf16, kind="ExternalInput")
    biT_d = nc.dram_tensor("biT", [1, 256], bf16, kind="ExternalInput")
    wfc_d = nc.dram_tensor("wfc", [128, 2 * IN_DIM], bf16, kind="ExternalInput")
    H0_d = nc.dram_tensor("H0", [128, 2 * T], bf16, kind="ExternalInput")
    c1a_d = nc.dram_tensor("c1a", [128, 1], f32, kind="ExternalInput")
    c1b_d = nc.dram_tensor("c1b", [128, 1], f32, kind="ExternalInput")
    yt_d = nc.dram_tensor("yt", [IN_DIM, T], f32, kind="ExternalOutput")

    # persistent SBUF
    wt_s = nc.alloc_sbuf_tensor("wt_s", [128, 2048], bf16)
    bias_s = nc.alloc_sbuf_tensor("bias_s", [128, 8], f32)
    wfc_s = nc.alloc_sbuf_tensor("wfc_s", [128, 2 * IN_DIM], bf16)
    H_s = nc.alloc_sbuf_tensor("H_s", [128, 2 * T], bf16)
    C_s = nc.alloc_sbuf_tensor("C_s", [128, 2 * T], f32)
    tg_s = nc.alloc_sbuf_tensor("tg_s", [128, 2 * N], f32)
    sif_s = nc.alloc_sbuf_tensor("sif_s", [128, 4 * N], f32)
    so_s = nc.alloc_sbuf_tensor("so_s", [128, 1024], f32)
    si_s = nc.alloc_sbuf_tensor("si_s", [128, 1024], f32)
    u_s = nc.alloc_sbuf_tensor("u_s", [128, 2 * N], f32)
    tc_s = nc.alloc_sbuf_tensor("tc_s", [128, 2 * N], f32)
    ysb = nc.alloc_sbuf_tensor("ysb", [IN_DIM, T], f32)
    warm_s = nc.alloc_sbuf_tensor("warm_s", [128, 640], bf16)
    boT_s = nc.alloc_sbuf_tensor("boT_s", [1, 256], bf16)
    biT_s = nc.alloc_sbuf_tensor("biT_s", [1, 256], bf16)
    ones_s = nc.alloc_sbuf_tensor("ones_s", [1, 512], bf16)

    wt_a = wt_s.ap()
    H_a = H_s.ap()
    C_a = C_s.ap()
    tg_a = tg_s.ap()
    sif_a = sif_s.ap()
    so_a = so_s.ap()
    si_a = si_s.ap()
    u_a = u_s.ap()
    tc_a = tc_s.ap()

    with tile.TileContext(nc) as tc_:
        nc.gpsimd.memset(warm_s.ap(), 0.0)
        nc.gpsimd.memset(ones_s.ap(), 1.0)
        nc.sync.dma_start(biT_s.ap(), biT_d.ap())
        nc.sync.dma_start(boT_s.ap(), boT_d.ap())
        nc.sync.dma_start(bias_s.ap(), bias_d.ap())
        nc.sync.dma_start(wt_a[:, 0:1024], wt_d.ap()[:, 0:1024])
        nc.sync.dma_start(wt_a[:, 1024:2048], wt_d.ap()[:, 1024:2048])
        nc.gpsimd.dma_start(H_a, H0_d.ap())
        # c1 -> C cols {0, T} (chunk-major position 0)
        nc.sync.dma_start(C_a[:, 0:1], c1a_d.ap())
        nc.sync.dma_start(C_a[:, T:T + 1], c1b_d.ap())
        nc.sync.dma_start(wfc_s.ap(), wfc_d.ap())

        with tc_.tile_pool(name="gates", bufs=1, space="PSUM") as gp:
            # Warm-up during the DMA phase: load the sigmoid/tanh ACT table
            # set, and stream zero-matmuls so the PE HAM clock-gate reaches
            # 8/8 before sweep 1 (a cold PE runs matmuls at half rate).
            nc.scalar.activation(tg_a[0:1, 0:1], warm_s.ap()[0:1, 0:1], SIG)
            wp7 = gp.tile([128, 1024], f32, tag="p67", name="wp7")
            for w in range(6):
                nc.tensor.matmul(wp7[:, 0:N], warm_s.ap()[:, 0:128],
                                 warm_s.ap()[:, 128:128 + N],
                                 start=True, stop=True)

            for s in range(nsweep):
                # g/f gates: 4 single-bank tiles with ACT-side bias;
                # i/o gates: 2-bank tiles (full 1024 cols) whose bias lands
                # via K=1 bf16 ones-matmuls, so each pair's sigmoid runs as
                # ONE contiguous ACT op with no bias operand.
                ps = {m: gp.tile([128, 512], f32, tag=f"p{m}", name=f"ps{m}")
                      for m in (0, 1, 4, 5)}
                ps23 = gp.tile([128, 1024], f32, tag="p23", name="ps23")
                ps67 = gp.tile([128, 1024], f32, tag="p67", name="ps67")
                # ones-matmuls have no H dependency: they fire during the
                # previous sweep's tail and double as PE warmkeepers.
                for pstile, bT in ((ps23, biT_s), (ps67, boT_s)):
                    for j in range(2):
                        nc.tensor.matmul(pstile[:, j * 512:(j + 1) * 512],
                                         bT.ap()[:, j * 128:(j + 1) * 128],
                                         ones_s.ap()[:, 0:512],
                                         start=True, stop=False,
                                         skip_group_check=True)
                # gates for positions 1..T-1 from H positions 0..T-2; bank
                # order matches the ACT order g0, i, f0, g1, f1, o
                order = [("g0", ps[0], N), ("i0", ps23, 512),
                         ("i1", ps23, 512), ("f0", ps[4], N),
                         ("g1", ps[1], N), ("f1", ps[5], N),
                         ("o0", ps67, 512), ("o1", ps67, 512)]
                off = {"i0": 0, "i1": 512, "o0": 0, "o1": 512}
                for q, (nm, pstile, nn) in enumerate(order):
                    for k in range(2):
                        if nm in off:
                            dst = pstile[:, off[nm]:off[nm] + nn]
                        else:
                            dst = pstile[:, 0:nn]
                        nc.tensor.matmul(
                            dst,
                            wt_a[:, q * 256 + k * 128:q * 256 + (k + 1) * 128],
                            H_a[:, k * T:k * T + nn],
                            start=(k == 0 and nm not in off), stop=(k == 1),
                            skip_group_check=True)
                # ACT pass 1 interleaved with the DVE cell path: the chunk-0
                # scan starts after only 3 ACT ops
                nc.scalar.activation(tg_a[:, 0:N], ps[0][:, 0:N], TANH,
                                     bias=bias_s.ap()[:, 0:1])
                nc.scalar.activation(si_a, ps23[:, 0:1024], SIG)
                nc.scalar.activation(sif_a[:, 2 * N:3 * N], ps[4][:, 0:N],
                                     SIG, bias=bias_s.ap()[:, 4:5])
                nc.vector.tensor_mul(u_a[:, 0:N], si_a[:, 0:N], tg_a[:, 0:N])
                nc.vector.tensor_tensor_scan(
                    C_a[:, 1:T], sif_a[:, 2 * N:3 * N], u_a[:, 0:N],
                    C_a[:, 0:1], ALU.mult, ALU.add)
                nc.scalar.activation(tg_a[:, N:2 * N], ps[1][:, 0:N], TANH,
                                     bias=bias_s.ap()[:, 1:2])
                nc.scalar.activation(sif_a[:, 3 * N:4 * N], ps[5][:, 0:N],
                                     SIG, bias=bias_s.ap()[:, 5:6])
                nc.vector.tensor_mul(u_a[:, N:2 * N], si_a[:, 512:512 + N],
                                     tg_a[:, N:2 * N])
                nc.vector.tensor_tensor_scan(
                    C_a[:, T + 1:2 * T], sif_a[:, 3 * N:4 * N],
                    u_a[:, N:2 * N], C_a[:, T:T + 1], ALU.mult, ALU.add)
                nc.scalar.activation(so_a, ps67[:, 0:1024], SIG)
                # tanh(c) then H = s_o * tanh(c) (bf16), split per chunk so
                # the next sweep's k=0 matmuls start as soon as chunk 0 of
                # H is written
                for k in range(2):
                    nc.scalar.activation(tc_a[:, k * N:(k + 1) * N],
                                         C_a[:, k * T + 1:(k + 1) * T], TANH)
                    nc.vector.tensor_mul(H_a[:, k * T + 1:(k + 1) * T],
                                         so_a[:, k * 512:k * 512 + N],
                                         tc_a[:, k * N:(k + 1) * N])
                # PE warmkeepers spread across the ACT/DVE tail (rhs deps
                # on u, C, tc stagger their fire times) so the PE clock-gate
                # never re-throttles between the sweeps' matmul phases
                for drhs in (u_a[:, 0:N], C_a[:, 1:T], tc_a[:, 0:N]):
                    nc.tensor.matmul(ps[0][0:8, 0:N], bias_s.ap(), drhs,
                                     start=True, stop=True,
                                     skip_group_check=True)

            # ---- output stage: y = W_fc @ H + b_fc  -> [23, T] ----
            y_ps = gp.tile([128, 512], f32, tag="p1", name="y_ps")
            for k in range(2):
                nc.tensor.matmul(y_ps[0:IN_DIM, 0:T],
                                 wfc_s.ap()[:, k * IN_DIM:(k + 1) * IN_DIM],
                                 H_a[:, k * T:(k + 1) * T],
                                 start=(k == 0), stop=(k == 1))
            nc.vector.tensor_copy(ysb.ap(), y_ps[0:IN_DIM, 0:T])
            nc.sync.dma_start(yt_d.ap(), ysb.ap())

    nc.compile()
    return nc


def kernel(feature, W_ih, W_hh, b_ih, b_hh, W_fc, b_fc, W_hfc, b_hfc):
    from concourse.bass_utils import run_bass_kernel_spmd

    per_core = _host_prep(feature, W_ih, W_hh, b_ih, b_hh, W_fc, b_fc,
                          W_hfc, b_hfc)

    if "nc" not in _CACHE:
        _CACHE["nc"] = build_program(SEQ_LEN, NSWEEP)
    nc = _CACHE["nc"]

    import os
    trace = bool(os.environ.get("LSTM_TRACE"))
    tmpdir = os.environ.get("LSTM_TRACE_DIR") or None
    res = run_bass_kernel_spmd(nc, per_core, list(range(BATCH)),
                               trace=trace, tmpdir=tmpdir)
    _CACHE["last_res"] = res
    bfc = np.asarray(b_fc, np.float32).reshape(1, IN_DIM)
    out = np.empty((BATCH, SEQ_LEN, IN_DIM), np.float32)
    for bb in range(BATCH):
        out[bb] = res.results[bb]["yt"].T + bfc
    return out


# revision 25
# speedup vs baseline: 1.0282x; 1.0282x over previous
"""Trainium2 Bass kernel for nn_Decoder_48859547959519.

Autoregressive LSTM decoder: 512 sequential steps, batch 8, hidden 256,
feedback y_t = fc(h_{t+1}) -> x_{t+1}.

Strategy: data parallel (1 batch element per NeuronCore, 8 cores) +
**parallel-in-time fixed-point iteration** instead of a serial 512-step
loop.

  * Algebraic fusion: x_{t+1} = W_fc h_{t+1} + b_fc  =>  for t >= 1
        gates_t = (W_ih W_fc + W_hh) h_t + (W_ih b_fc + b) = W_eff h_t + b_eff
    so each trajectory position needs one 256->1024 matvec + LSTM cell.
    Step 0 (x_0 = 0) is peeled on the host.
  * The whole trajectory H = [h_1 .. h_512] is iterated as a fixed point:
        gates^k  = W_eff H^{k-1}(shifted) + b     (16 batched N~512 matmuls)
        i,f,o,g  = sigmoid/tanh(gates^k)          (big ACT ops)
        c^k      = exact scan: c_t = f_t*c_{t-1} + i_t*tanh(g_t)
                                                  (DVE tensor_tensor_scan!)
        H^k      = o^k * tanh(c^k)
    Given the gates, the c-recurrence is solved EXACTLY within a sweep by
    the hardware prefix-scan; only the h-feedback lags one sweep.  The
    step map is strongly contractive: one sweep is host-precomputed in
    closed form as the initial guess (H^0 is zero beyond position 0, so
    its gates need no trajectory matmul), and NSWEEP=2 device sweeps
    reach rel err ~5.8e-3 vs the 2e-2 gate (3 sweeps -> 2.3e-3).
  * ACT is the per-sweep bottleneck, so the i- and o-gate biases land in
    PSUM via K=1 bf16 ones-matmuls, letting each pair's sigmoid run as a
    single contiguous 1024-col ACT op with no bias operand; g/f keep
    per-chunk ACT-side per-partition biases.  ACT order g0,i,f0,g1,f1
    lets the chunk-0 c-scan start after three ACT ops while the rest of
    pass 1 continues.
  * The PE clock-gate (HAM) runs matmuls at half rate until ~3.4us of
    sustained activity, and re-throttles during idle gaps: zero-matmuls
    warm it up during the initial DMAs, and dummy matmuls whose operand
    dependencies (u, C, tc) stagger them across each sweep's ACT/DVE
    tail keep it warm between the sweeps' matmul phases.
  * Every op is a big batched op (N=511..1024) so fixed instruction
    overheads amortize; there is no per-timestep serial chain at all.
"""

import numpy as np

SEQ_LEN = 512
IN_DIM = 23
HID = 256
FEAT = 128
BATCH = 8
NCHUNK = 8  # 4*HID / 128
# chunk order: [g0 g1 i0 i1 f0 f1 o0 o1]
# (PyTorch gate-row order in W_eff is i:0 f:256 g:512 o:768)
CHUNK_ROWS = [512, 640, 0, 128, 256, 384, 768, 896]
# bank emission order within a sweep: g0, i0, i1, f0, g1, f1, o0, o1
MM_ORDER = [0, 2, 3, 4, 1, 5, 6, 7]
NSWEEP = 2

_CACHE = {}


def _sigmoid(x):
    return 1.0 / (1.0 + np.exp(-x))


def _host_prep(feature, W_ih, W_hh, b_ih, b_hh, W_fc, b_fc, W_hfc, b_hfc):
    """Fuse the feedback path, peel step 0, pack device tensors."""
    f32 = np.float32
    W_ih = np.asarray(W_ih, f32)
    W_hh = np.asarray(W_hh, f32)
    W_fc = np.asarray(W_fc, f32)
    b = np.asarray(b_ih, f32) + np.asarray(b_hh, f32)

    W_eff = (W_ih @ W_fc + W_hh).astype(f32)          # [1024, 256]
    b_eff = (W_ih @ np.asarray(b_fc, f32) + b).astype(f32)  # [1024]

    # step 0 on host (x_0 = 0): h0 from feature, c0 = 0
    feats = np.asarray(feature, f32)                  # [B, FEAT]
    h0 = feats @ np.asarray(W_hfc, f32).T + np.asarray(b_hfc, f32)  # [B, HID]
    g0 = h0 @ W_hh.T + b                              # [B, 1024]
    i_g, f_g, g_g, o_g = np.split(g0, 4, axis=1)
    c1 = _sigmoid(i_g) * np.tanh(g_g)                 # [B, HID]
    h1 = _sigmoid(o_g) * np.tanh(c1)                  # [B, HID]

    # pack weight tiles bank-major in matmul issue order (g0,i0,i1,f0,
    # g1,f1,o0,o1), both k-tiles of a bank adjacent, so the upload can be
    # split into two contiguous DMAs that deliver the first banks early:
    # wt[p, q*256 + k*128 + j] = W_eff[row(MM_ORDER[q])+j, k*128+p]
    wt = np.empty((128, 2048), np.float32)
    for q, m in enumerate(MM_ORDER):
        for k in range(2):
            blk = W_eff[CHUNK_ROWS[m]:CHUNK_ROWS[m] + 128,
                        k * 128:(k + 1) * 128]        # [j, p]
            wt[:, q * 256 + k * 128:q * 256 + (k + 1) * 128] = blk.T
    # per-chunk bias as [128, 8] per-partition vectors (ACT bias operand)
    bias_sb = np.stack([b_eff[r:r + 128] for r in CHUNK_ROWS], 1)  # [128, 8]

    # fc weights for the output stage: wfc[p, k*23+d] = W_fc[d, k*128+p]
    wfc = np.empty((128, 2 * IN_DIM), np.float32)
    for k in range(2):
        wfc[:, k * IN_DIM:(k + 1) * IN_DIM] = W_fc[:, k * 128:(k + 1) * 128].T
    bfc = np.asarray(b_fc, f32).reshape(IN_DIM, 1)

    import ml_dtypes
    bf16 = ml_dtypes.bfloat16
    T = SEQ_LEN
    per_core = []
    for bb in range(BATCH):
        # Initial guess = device sweep 1 computed in closed form on the host:
        # H^0 is zero except position 0 (= h1), so sweep-1 gates are
        # W_eff h1 + b at position 1 and plain b elsewhere -- one matvec
        # plus a scalar recurrence.  (Equivalent to one device sweep, in
        # fp32; the device then runs NSWEEP real sweeps on top.)
        H0 = np.zeros((128, 2 * T), np.float32)
        H0[:, 0] = h1[bb, 0:128]
        H0[:, T] = h1[bb, 128:256]
        c1p = np.stack([c1[bb, 0:128], c1[bb, 128:256]], 1)  # [128, 2]
        g1v = W_eff @ h1[bb] + b_eff                  # gates at position 1
        gbv = b_eff                                   # gates at positions >= 2
        ii, ff, gg, oo = (slice(0, 256), slice(256, 512),
                          slice(512, 768), slice(768, 1024))
        u1 = _sigmoid(g1v[ii]) * np.tanh(g1v[gg])
        ub = _sigmoid(gbv[ii]) * np.tanh(gbv[gg])
        f1 = _sigmoid(g1v[ff])
        fb = _sigmoid(gbv[ff])
        o1 = _sigmoid(g1v[oo])
        ob = _sigmoid(gbv[oo])
        cj = np.concatenate([c1p[:, 0], c1p[:, 1]])   # c at position 0
        Hf = np.zeros((256, T), np.float32)
        Hf[:, 0] = h1[bb]
        for t in range(1, T):
            cj = (f1 if t == 1 else fb) * cj + (u1 if t == 1 else ub)
            Hf[:, t] = (o1 if t == 1 else ob) * np.tanh(cj)
        H0[:, 0:T] = Hf[0:128]
        H0[:, T:2 * T] = Hf[128:256]
        per_core.append({
            "wt": wt.astype(bf16),
            "bias": bias_sb.astype(f32),
            "boT": np.concatenate([b_eff[768:896], b_eff[896:1024]]
                                  ).reshape(1, 256).astype(bf16),
            "biT": np.concatenate([b_eff[0:128], b_eff[128:256]]
                                  ).reshape(1, 256).astype(bf16),
            "wfc": wfc.astype(bf16),
            "H0": H0.astype(bf16),
            "c1a": np.ascontiguousarray(c1p[:, 0:1]),
            "c1b": np.ascontiguousarray(c1p[:, 1:2]),
        })
    return per_core


def build_program(T=SEQ_LEN, nsweep=NSWEEP):
    """Emit the Bass/Tile program (fully static, no hardware loop)."""
    import concourse.bacc as bacc
    import concourse.mybir as mybir
    import concourse.tile as tile

    f32 = mybir.dt.float32
    bf16 = mybir.dt.bfloat16
    SIG = mybir.ActivationFunctionType.Sigmoid
    TANH = mybir.ActivationFunctionType.Tanh
    IDT = mybir.ActivationFunctionType.Identity
    ALU = mybir.AluOpType

    N = T - 1  # positions computed per sweep (pos 1..T-1); pos 0 fixed
    nc = bacc.Bacc("TRN2", target_bir_lowering=False, debug=False)

    # DRAM I/O
    wt_d = nc.dram_tensor("wt", [128, 2048], bf16, kind="ExternalInput")
    bias_d = nc.dram_tensor("bias", [128, 8], f32, kind="ExternalInput")
    boT_d = nc.dram_tensor("boT", [1, 256], bf16, kind="ExternalInput")
    biT_d = nc.dram_tensor("biT", [1, 256], bf16, kind="ExternalInput")
    wfc_d = nc.dram_tensor("wfc", [128, 2 * IN_DIM], bf16, kind="ExternalInput")
    H0_d = nc.dram_tensor("H0", [128, 2 * T], bf16, kind="ExternalInput")
    c1a_d = nc.dram_tensor("c1a", [128, 1], f32, kind="ExternalInput")
    c1b_d = nc.dram_tensor("c1b", [128, 1], f32, kind="ExternalInput")
    yt_d = nc.dram_tensor("yt", [IN_DIM, T], f32, kind="ExternalOutput")

    # persistent SBUF
    wt_s = nc.alloc_sbuf_tensor("wt_s", [128, 2048], bf16)
    bias_s = nc.alloc_sbuf_tensor("bias_s", [128, 8], f32)
    wfc_s = nc.alloc_sbuf_tensor("wfc_s", [128, 2 * IN_DIM], bf16)
    H_s = nc.alloc_sbuf_tensor("H_s", [128, 2 * T], bf16)
    C_s = nc.alloc_sbuf_tensor("C_s", [128, 2 * T], f32)
    tg_s = nc.alloc_sbuf_tensor("tg_s", [128, 2 * N], f32)
    sif_s = nc.alloc_sbuf_tensor("sif_s", [128, 4 * N], f32)
    so_s = nc.alloc_sbuf_tensor("so_s", [128, 1024], f32)
    si_s = nc.alloc_sbuf_tensor("si_s", [128, 1024], f32)
    u_s = nc.alloc_sbuf_tensor("u_s", [128, 2 * N], f32)
    tc_s = nc.alloc_sbuf_tensor("tc_s", [128, 2 * N], f32)
    ysb = nc.alloc_sbuf_tensor("ysb", [IN_DIM, T], f32)
    warm_s = nc.alloc_sbuf_tensor("warm_s", [128, 640], bf16)
    boT_s = nc.alloc_sbuf_tensor("boT_s", [1, 256], bf16)
    biT_s = nc.alloc_sbuf_tensor("biT_s", [1, 256], bf16)
    ones_s = nc.alloc_sbuf_tensor("ones_s", [1, 512], bf16)

    wt_a = wt_s.ap()
    H_a = H_s.ap()
    C_a = C_s.ap()
    tg_a = tg_s.ap()
    sif_a = sif_s.ap()
    so_a = so_s.ap()
    si_a = si_s.ap()
    u_a = u_s.ap()
    tc_a = tc_s.ap()

    with tile.TileContext(nc) as tc_:
        nc.gpsimd.memset(warm_s.ap(), 0.0)
        nc.gpsimd.memset(ones_s.ap(), 1.0)
        nc.sync.dma_start(biT_s.ap(), biT_d.ap())
        nc.sync.dma_start(boT_s.ap(), boT_d.ap())
        nc.sync.dma_start(bias_s.ap(), bias_d.ap())
        nc.sync.dma_start(wt_a[:, 0:1024], wt_d.ap()[:, 0:1024])
        nc.sync.dma_start(wt_a[:, 1024:2048], wt_d.ap()[:, 1024:2048])
        nc.gpsimd.dma_start(H_a, H0_d.ap())
        # c1 -> C cols {0, T} (chunk-major position 0)
        nc.sync.dma_start(C_a[:, 0:1], c1a_d.ap())
        nc.sync.dma_start(C_a[:, T:T + 1], c1b_d.ap())
        nc.sync.dma_start(wfc_s.ap(), wfc_d.ap())

        with tc_.tile_pool(name="gates", bufs=1, space="PSUM") as gp:
            # Warm-up during the DMA phase: load the sigmoid/tanh ACT table
            # set, and stream zero-matmuls so the PE HAM clock-gate reaches
            # 8/8 before sweep 1 (a cold PE runs matmuls at half rate).
            nc.scalar.activation(tg_a[0:1, 0:1], warm_s.ap()[0:1, 0:1], SIG)
            wp7 = gp.tile([128, 1024], f32, tag="p67", name="wp7")
            for w in range(6):
                nc.tensor.matmul(wp7[:, 0:N], warm_s.ap()[:, 0:128],
                                 warm_s.ap()[:, 128:128 + N],
                                 start=True, stop=True)

            for s in range(nsweep):
                # g/f gates: 4 single-bank tiles with ACT-side bias;
                # i/o gates: 2-bank tiles (full 1024 cols) whose bias lands
                # via K=1 bf16 ones-matmuls, so each pair's sigmoid runs as
                # ONE contiguous ACT op with no bias operand.
                ps = {m: gp.tile([128, 512], f32, tag=f"p{m}", name=f"ps{m}")
                      for m in (0, 1, 4, 5)}
                ps23 = gp.tile([128, 1024], f32, tag="p23", name="ps23")
                ps67 = gp.tile([128, 1024], f32, tag="p67", name="ps67")
                # ones-matmuls have no H dependency: they fire during the
                # previous sweep's tail and double as PE warmkeepers.
                for pstile, bT in ((ps23, biT_s), (ps67, boT_s)):
                    for j in range(2):
                        nc.tensor.matmul(pstile[:, j * 512:(j + 1) * 512],
                                         bT.ap()[:, j * 128:(j + 1) * 128],
                                         ones_s.ap()[:, 0:512],
                                         start=True, stop=False,
                                         skip_group_check=True)
                # gates for positions 1..T-1 from H positions 0..T-2; bank
                # order matches the ACT order g0, i, f0, g1, f1, o
                order = [("g0", ps[0], N), ("i0", ps23, 512),
                         ("i1", ps23, 512), ("f0", ps[4], N),
                         ("g1", ps[1], N), ("f1", ps[5], N),
                         ("o0", ps67, 512), ("o1", ps67, 512)]
                off = {"i0": 0, "i1": 512, "o0": 0, "o1": 512}
                for q, (nm, pstile, nn) in enumerate(order):
                    for k in range(2):
                        if nm in off:
                            dst = pstile[:, off[nm]:off[nm] + nn]
                        else:
                            dst = pstile[:, 0:nn]
                        nc.tensor.matmul(
                            dst,
                            wt_a[:, q * 256 + k * 128:q * 256 + (k + 1) * 128],
                            H_a[:, k * T:k * T + nn],
                            start=(k == 0 and nm not in off), stop=(k == 1),
                            skip_group_check=True)
                # ACT pass 1 interleaved with the DVE cell path: the chunk-0
                # scan starts after only 3 ACT ops
                nc.scalar.activation(tg_a[:, 0:N], ps[0][:, 0:N], TANH,
                                     bias=bias_s.ap()[:, 0:1])
                nc.scalar.activation(si_a, ps23[:, 0:1024], SIG)
                nc.scalar.activation(sif_a[:, 2 * N:3 * N], ps[4][:, 0:N],
                                     SIG, bias=bias_s.ap()[:, 4:5])
                nc.vector.tensor_mul(u_a[:, 0:N], si_a[:, 0:N], tg_a[:, 0:N])
                nc.vector.tensor_tensor_scan(
                    C_a[:, 1:T], sif_a[:, 2 * N:3 * N], u_a[:, 0:N],
                    C_a[:, 0:1], ALU.mult, ALU.add)
                nc.scalar.activation(tg_a[:, N:2 * N], ps[1][:, 0:N], TANH,
                                     bias=bias_s.ap()[:, 1:2])
                nc.scalar.activation(sif_a[:, 3 * N:4 * N], ps[5][:, 0:N],
                                     SIG, bias=bias_s.ap()[:, 5:6])
                nc.vector.tensor_mul(u_a[:, N:2 * N], si_a[:, 512:512 + N],
                                     tg_a[:, N:2 * N])
                nc.vector.tensor_tensor_scan(
                    C_a[:, T + 1:2 * T], sif_a[:, 3 * N:4 * N],
                    u_a[:, N:2 * N], C_a[:, T:T + 1], ALU.mult, ALU.add)
                nc.scalar.activation(so_a, ps67[:, 0:1024], SIG)
                # tanh(c) then H = s_o * tanh(c) (bf16), split per chunk so
                # the next sweep's k=0 matmuls start as soon as chunk 0 of
                # H is written
                for k in range(2):
                    nc.scalar.activation(tc_a[:, k * N:(k + 1) * N],
                                         C_a[:, k * T + 1:(k + 1) * T], TANH)
                    nc.vector.tensor_mul(H_a[:, k * T + 1:(k + 1) * T],
                                         so_a[:, k * 512:k * 512 + N],
                                         tc_a[:, k * N:(k + 1) * N])
                # PE warmkeepers spread across the ACT/DVE tail (rhs deps
                # on u, C, tc stagger their fire times) so the PE clock-gate
                # never re-throttles between the sweeps' matmul phases
                for drhs in (u_a[:, 0:N], C_a[:, 1:T], tc_a[:, 0:N]):
                    nc.tensor.matmul(ps[0][0:8, 0:N], bias_s.ap(), drhs,
                                     start=True, stop=True,
                                     skip_group_check=True)

            # ---- output stage: y = W_fc @ H + b_fc  -> [23, T] ----
            y_ps = gp.tile([128, 512], f32, tag="p1", name="y_ps")
            for k in range(2):
                nc.tensor.matmul(y_ps[0:IN_DIM, 0:T],
                                 wfc_s.ap()[:, k * IN_DIM:(k + 1) * IN_DIM],
                                 H_a[:, k * T:(k + 1) * T],
                                 start=(k == 0), stop=(k == 1))
            nc.vector.tensor_copy(ysb.ap(), y_ps[0:IN_DIM, 0:T])
            nc.sync.dma_start(yt_d.ap(), ysb.ap())

    nc.compile()
    return nc


def kernel(feature, W_ih, W_hh, b_ih, b_hh, W_fc, b_fc, W_hfc, b_hfc):
    from concourse.bass_utils import run_bass_kernel_spmd

    per_core = _host_prep(feature, W_ih, W_hh, b_ih, b_hh, W_fc, b_fc,
                          W_hfc, b_hfc)

    if "nc" not in _CACHE:
        _CACHE["nc"] = build_program(SEQ_LEN, NSWEEP)
    nc = _CACHE["nc"]

    import os
    trace = bool(os.environ.get("LSTM_TRACE"))
    tmpdir = os.environ.get("LSTM_TRACE_DIR") or None
    res = run_bass_kernel_spmd(nc, per_core, list(range(BATCH)),
                               trace=trace, tmpdir=tmpdir)
    _CACHE["last_res"] = res
    bfc = np.asarray(b_fc, np.float32).reshape(1, IN_DIM)
    out = np.empty((BATCH, SEQ_LEN, IN_DIM), np.float32)
    for bb in range(BATCH):
        out[bb] = res.results[bb]["yt"].T + bfc
    return out
